# revision 37
# baseline (speedup 1.0000x reference)
"""MoE (DeepSeek-style, no gate) SwiGLU kernel for 8 Trainium2 NeuronCores.

Strategy (expert parallelism, per the sharding hint):
  - 16 routed experts sharded 2-per-core across 8 cores.
  - Token dispatch done host-side: for each expert, gather the tokens routed
    to it (topk membership), pad to a common capacity C, and ship the
    pre-transposed activation columns xT[:, tokens] to the owning core.
  - Shared expert tensor-parallel over its inter dim (2816/8 = 352 cols per
    core, zero-padded to 384), computed on all 2048 tokens in 2 chunks.
  - Each core runs the same Bass program (SPMD) on its own shard; the host
    scatter-adds routed outputs and sums shared-expert partials.

Compute layout per phase (one phase = one SwiGLU MLP on a token set):
  phase 1:  h1T = W1p.T @ xT, h3T = W3p.T @ xT   (I on partitions, tokens free)
            h' = silu(h1T) * h3T                  -> bf16 in SBUF
  phase 2:  y[tok_tile] = h'.T @ W2p, scaled per-token by comb weight on
            PSUM eviction (tensor_tensor with a broadcast comb row).

All weights/activations are cast to bf16 on host (halves HBM traffic; PE
runs bf16 at 1 cycle/row). PSUM accumulation is fp32; outputs are written
bf16 (partition-major [P, kd, C] so DMA rows are multi-KB) and upcast on
the host. Host pre-tiles every tensor so that every DMA is fully contiguous.

Schedule notes (from trace analysis of the f32-out baseline):
  - The PE matmul stream is ~97% dense at ~2.37GHz; the recoverable time is
    startup (first-phase input load through one DMA queue) and the output
    drain after the last matmul.
  - Startup: the first phase's xt + cb ride the ACT HWDGE ring (idle until
    the first PSUM group lands) in parallel with w13 on the SP ring, and the
    first W13 I-tile panel is split in two so the first matmul's dependency
    is a 0.5MB DMA, not 1MB. Later phases keep everything on the SP ring
    (ACT-ring DMAs would block the silu/copy stream behind them).
  - Tail: outputs are grouped 4 D-tiles per DMA ([P, 4, C] bf16, 5.5KB rows
    instead of 2.7KB f32 rows) and the last phase ends with 2-tile groups so
    the final eviction+DMA after the last matmul is small.
"""

import numpy as np
import ml_dtypes

import concourse.bass as bass
import concourse.bacc as bacc
import concourse.mybir as mybir
import concourse.tile as tile
from concourse.bass_utils import run_bass_kernel_spmd

BF16 = ml_dtypes.bfloat16
F32 = np.float32
P = 128
NSZ = 512  # PSUM bank free size (fp32)
XG = 4    # xt k-tiles per DMA (HWDGE executes DMAs serially; batch them)

FULL_CFG = dict(
    ncores=8,
    T=2048,
    D=2048,
    E=16,
    I_E=1408,
    sh_half=1408,    # shared expert sharded 2 (inter) x ncores/2 (tokens)
    d_out=2048,
)


def _derived(cfg):
    nt = max(1, cfg["ncores"] // 2)
    return dict(
        epc=cfg["E"] // cfg["ncores"],
        kd=cfg["D"] // P,
        it_r=cfg["I_E"] // P,
        it_s=cfg["sh_half"] // P,
        n_tok_shards=nt,
        sh_tok=cfg["T"] // nt,
    )


def _out_groups(kd, last_phase):
    """D-tile grouping for output DMAs. 4-tile groups (5.5KB bf16 rows);
    the last phase ends with 2-tile groups so the post-matmul drain is
    small."""
    if last_phase:
        gs = [4] * ((kd - 4) // 4) + [2, 1, 1]
    else:
        gs = [4] * (kd // 4)
    assert sum(gs) == kd
    return gs


def _emit_phase(nc, pools, xt_dram, w13_dram, w13_split, w2_dram, cb_dram,
                out_dram, n_itiles, cp, cfg, ph, fast_start=False,
                last_phase=False):
    """One SwiGLU MLP phase over `cp` token columns with `n_itiles` I-tiles.

    out_dram is [P, kd, cp] bf16 (partition-major) so each group DMA writes
    multi-KB contiguous rows. cb_dram is None for the shared expert
    (no per-token combine weight); otherwise it is the combine weight
    broadcast to [P, cp]. When fast_start is set, xt + cb load via the ACT
    HWDGE ring (in parallel with w13 on the SP ring) and w13_split carries
    the first I-tile's panel as two [P, 2, kd/2, P] halves.
    """
    kd = _derived(cfg)["kd"]
    d_out = cfg["d_out"]
    dt = mybir.dt.bfloat16
    f32 = mybir.dt.float32

    xtp, wp, wsp, hpp, w2p, sp, op, cgp, psA, psY = (
        pools["xt"], pools["w"], pools["ws"], pools["hp"], pools["w2"],
        pools["s"], pools["o"], pools["cg"], pools["psA"], pools["psY"])

    # DMA order, all on the SP ring (the SP and ACT HWDGE queues share the
    # same ~395B/ns bandwidth, so a second ring buys nothing and reorders
    # the critical stream). Fast-start phase: interleave finest-dependency
    # first — [w13 half 0, xt g0, w13 half 1, xt g1..g3] — so the first
    # matmul's dependency is ~1.2MB, not 3.8MB.
    wpre = []
    xtg = []
    if fast_start:
        # The startup ramp is DMA-bandwidth-bound, so order the stream by
        # need: w13(m0) halves + the FIRST TOKEN HALF of every xt group
        # (chunk 0 of I-tile 0 computes on these while the second halves
        # stream in), then the second halves, then the w13(m1) halves (a
        # full m=1 panel queued behind xt arrives ~1us late otherwise).
        # m=0 runs 256-token PSUM chunks to match (see nsz_m below).
        half = cp // 2
        wh0 = wsp.tile([P, 2, kd // 2, P], dt, tag="w13a", name=f"w13a_{ph}")
        wh1 = wsp.tile([P, 2, kd // 2, P], dt, tag="w13b", name=f"w13b_{ph}")
        for g in range(kd // XG):
            xtg.append(xtp.tile([P, XG, cp], dt, tag=f"xt_{g}",
                                name=f"xt_{ph}_{g}"))
        nc.sync.dma_start(out=wh0[:], in_=w13_split[0])
        nc.sync.dma_start(out=xtg[0][:, :, 0:half], in_=xt_dram[0][0])
        nc.sync.dma_start(out=wh1[:], in_=w13_split[1])
        for g in range(1, kd // XG):
            nc.sync.dma_start(out=xtg[g][:, :, 0:half], in_=xt_dram[g][0])
        for g in range(kd // XG):
            nc.sync.dma_start(out=xtg[g][:, :, half:cp], in_=xt_dram[g][1])
        wpre.append((wh0, wh1))
    else:
        w0 = wp.tile([P, 2, kd, P], dt, tag="w13", name=f"w13_{ph}_0")
        nc.sync.dma_start(out=w0[:], in_=w13_dram[0])
        wpre.append(w0)
    for g in range(len(xtg), kd // XG):
        xge = xtp.tile([P, XG, cp], dt, tag=f"xt_{g}", name=f"xt_{ph}_{g}")
        nc.sync.dma_start(out=xge[:], in_=xt_dram[g])
        xtg.append(xge)
    assert len(xtg) == kd // XG
    if n_itiles > 1:
        if fast_start:
            w1h0 = wsp.tile([P, 2, kd // 2, P], dt, tag="w13c", name=f"w13c_{ph}")
            w1h1 = wsp.tile([P, 2, kd // 2, P], dt, tag="w13d", name=f"w13d_{ph}")
            nc.sync.dma_start(out=w1h0[:], in_=w13_split[2])
            nc.sync.dma_start(out=w1h1[:], in_=w13_split[3])
            wpre.append((w1h0, w1h1))
        else:
            w13b = wp.tile([P, 2, kd, P], dt, tag="w13", name=f"w13_{ph}_1")
            nc.sync.dma_start(out=w13b[:], in_=w13_dram[1])
            wpre.append(w13b)

    cbt = None
    if cb_dram is not None:
        cbr = cgp.tile([P, cp], f32, tag="cbr", name=f"cbr_{ph}")
        nc.sync.dma_start(out=cbr[:], in_=cb_dram[:])
        # Bounce through DVE so the per-tile eviction muls below need only
        # the PE wait (DVE has already observed the cb DMA here).
        cbt = cgp.tile([P, cp], f32, tag="cb", name=f"cb_{ph}")
        nc.vector.tensor_copy(cbt[:], cbr[:])

    def w13_slice(m, w, kt):
        if fast_start and m < 2:
            wh0, wh1 = wpre[m]
            h = kd // 2
            return (wh0 if kt < h else wh1)[:, w, kt % h, :]
        w13t = wpre[m] if m < len(wpre) else wcur[0]
        return w13t[:, w, kt, :]

    # W2 panel tile: loaded in per-I-tile chunk DMAs interleaved into the
    # phase-1 loop (subtile deps let each phase-2 matmul wait only on its
    # k-slice). A single 5.8MB DMA queued after all phase-1 loads arrives
    # just-in-time and stalls the first phase-2 groups; chunks fill the
    # ring's buffer-gated idle gaps instead.
    w2t = w2p.tile([P, n_itiles, d_out], dt, tag="w2", name=f"w2_{ph}")

    def w2_chunk(kt):
        nc.sync.dma_start(out=w2t[:, kt, :], in_=w2_dram[:, kt, :])

    # ---- phase 1: h' = silu(xW1) * (xW3), transposed layout [I, tokens] ----
    hp = []
    wcur = [None]
    w2_chunk(0)
    w2_chunk(1)
    for m in range(n_itiles):
        if m >= len(wpre):
            wcur[0] = wp.tile([P, 2, kd, P], dt, tag="w13", name=f"w13_{ph}_{m}")
            nc.sync.dma_start(out=wcur[0][:], in_=w13_dram[m])
            if m < n_itiles - 1:
                w2_chunk(m)
        if m == n_itiles - 1:
            w2_chunk(n_itiles - 1)
        hpm = hpp.tile([P, cp], dt, tag=f"hp_{m}", name=f"hp_{ph}_{m}")
        # Fast phase, first I-tile: half-size chunks so chunk-0 compute
        # overlaps the second token half's DMA stream.
        nsz_m = cp // 2 if (fast_start and m == 0) else NSZ
        for n0 in range(0, cp, nsz_m):
            nsz = min(nsz_m, cp - n0)
            p1 = psA.tile([P, nsz], f32, tag="p1", name=f"p1_{ph}_{m}_{n0}")
            p3 = psA.tile([P, nsz], f32, tag="p3", name=f"p3_{ph}_{m}_{n0}")
            for kt in range(kd):
                nc.tensor.matmul(p1[:], w13_slice(m, 0, kt),
                                 xtg[kt // XG][:, kt % XG, n0:n0 + nsz],
                                 start=(kt == 0), stop=(kt == kd - 1))
            for kt in range(kd):
                nc.tensor.matmul(p3[:], w13_slice(m, 1, kt),
                                 xtg[kt // XG][:, kt % XG, n0:n0 + nsz],
                                 start=(kt == 0), stop=(kt == kd - 1))
            # silu(h1)*h3: ACT Silu LUT + ACT copy read PSUM (wait on PE);
            # the DVE mul then waits on ACT only — the DVE TensorTensor
            # encoding only has room for one sync-wait command.
            s = sp.tile([P, nsz], f32, tag="s", name=f"s_{ph}_{m}_{n0}")
            nc.scalar.activation(s[:], p1[:],
                                 mybir.ActivationFunctionType.Silu)
            c3 = sp.tile([P, nsz], f32, tag="c3", name=f"c3_{ph}_{m}_{n0}")
            nc.scalar.copy(c3[:], p3[:])
            nc.vector.tensor_mul(hpm[:, n0:n0 + nsz], s[:], c3[:])
        hp.append(hpm)

    # ---- phase 2: out[tok] = comb * (h'.T @ W2) ----
    # Cycle PSUM tags so phase 2 rotates through all 8 banks (phase 1's
    # p1/p3 slots are idle here).
    ps2 = [(psY, "py"), (psY, "py"), (psY, "py"), (psY, "py"),
           (psA, "p1"), (psA, "p1"), (psA, "p3"), (psA, "p3")]
    idx = 0
    mt2 = 0
    for gi, gsz in enumerate(_out_groups(kd, last_phase)):
        osb = op.tile([P, gsz, cp], dt, tag="osb", name=f"osb_{ph}_{gi}")
        for j in range(gsz):
            for n0 in range(0, cp, NSZ):
                nn = min(NSZ, cp - n0)
                pool, ptag = ps2[idx % len(ps2)]
                idx += 1
                py = pool.tile([P, nn], f32, tag=ptag,
                               name=f"py_{ph}_{mt2}_{n0}")
                for kt in range(n_itiles):
                    nc.tensor.matmul(py[:], w2t[:, kt, mt2 * P:(mt2 + 1) * P],
                                     hp[kt][:, n0:n0 + nn],
                                     start=(kt == 0), stop=(kt == n_itiles - 1))
                if cbt is not None:
                    nc.vector.tensor_mul(osb[:, j, n0:n0 + nn], py[:],
                                         cbt[:, n0:n0 + nn])
                elif idx % 2:
                    # Shared-expert evictions alternate DVE/ACT: a single
                    # engine's copy throughput can't keep up with PE and PE
                    # stalls on bank recycling.
                    nc.vector.tensor_copy(osb[:, j, n0:n0 + nn], py[:])
                else:
                    nc.scalar.copy(osb[:, j, n0:n0 + nn], py[:])
            mt2 += 1
        nc.sync.dma_start(out=out_dram[:, mt2 - gsz:mt2, :], in_=osb[:])


def build_program(Cs, cfg):
    """Build the per-core Bass program. Cs[j] = token capacity of routed
    expert slot j (experts are sorted by routed-token count into slots, so
    each slot's capacity matches its own worst case)."""
    nc = bacc.Bacc()
    dt = mybir.dt.bfloat16
    f32 = mybir.dt.float32
    dv = _derived(cfg)
    epc, kd, it_r, it_s = dv["epc"], dv["kd"], dv["it_r"], dv["it_s"]
    sh_tok = dv["sh_tok"]
    d_out = cfg["d_out"]

    ins = {}
    for j in range(epc):
        ins[f"xt{j}"] = nc.dram_tensor(f"xt{j}", [kd // XG, P, XG, Cs[j]], dt, kind="ExternalInput")
        ins[f"w13_{j}"] = nc.dram_tensor(f"w13_{j}", [it_r, P, 2, kd, P], dt, kind="ExternalInput")
        ins[f"w2_{j}"] = nc.dram_tensor(f"w2_{j}", [P, it_r, d_out], dt, kind="ExternalInput")
        ins[f"cb{j}"] = nc.dram_tensor(f"cb{j}", [P, Cs[j]], f32, kind="ExternalInput")
    # Fast-start split of the shared expert's first W13 I-tile panel (the
    # shared phase runs first: it has the smallest startup-critical load).
    ins["ws13s"] = nc.dram_tensor(
        "ws13s", [4, P, 2, kd // 2, P], dt, kind="ExternalInput")
    # Fast phase's xt: token-halved so chunk-0 only depends on half the load.
    ins["xts"] = nc.dram_tensor("xts", [kd // XG, 2, P, XG, sh_tok // 2], dt, kind="ExternalInput")
    ins["ws13"] = nc.dram_tensor("ws13", [it_s, P, 2, kd, P], dt, kind="ExternalInput")
    ins["ws2"] = nc.dram_tensor("ws2", [P, it_s, d_out], dt, kind="ExternalInput")

    # Outputs are partition-major [P, kd, tokens] bf16.
    outs = {}
    for j in range(epc):
        outs[f"y{j}"] = nc.dram_tensor(f"y{j}", [P, kd, Cs[j]], dt, kind="ExternalOutput")
    outs["z"] = nc.dram_tensor("z", [P, kd, sh_tok], dt, kind="ExternalOutput")

    with tile.TileContext(nc) as tc:
        with (
            tc.tile_pool(name="xt", bufs=2) as xtp,
            tc.tile_pool(name="w", bufs=3) as wp,
            tc.tile_pool(name="ws", bufs=1) as wsp,
            tc.tile_pool(name="hp", bufs=1) as hpp,
            tc.tile_pool(name="w2", bufs=1) as w2p,
            tc.tile_pool(name="s", bufs=2) as sp,
            tc.tile_pool(name="o", bufs=2) as op,
            tc.tile_pool(name="cg", bufs=1) as cgp,
            tc.tile_pool(name="psA", bufs=2, space="PSUM") as psA,
            tc.tile_pool(name="psY", bufs=4, space="PSUM") as psY,
        ):
            pools = dict(xt=xtp, w=wp, ws=wsp, hp=hpp, w2=w2p, s=sp, o=op,
                         cg=cgp, psA=psA, psY=psY)
            # Shared phase first (smallest startup-critical load: 2.1MB xts,
            # no cb), then experts, smaller slot before larger.
            _emit_phase(nc, pools, ins["xts"], ins["ws13"], ins["ws13s"],
                        ins["ws2"], None, outs["z"], it_s, sh_tok, cfg,
                        ph="s", fast_start=True)
            slot_order = sorted(range(epc), key=lambda j: Cs[j])
            for i, j in enumerate(slot_order):
                _emit_phase(nc, pools, ins[f"xt{j}"], ins[f"w13_{j}"],
                            None, ins[f"w2_{j}"], ins[f"cb{j}"],
                            outs[f"y{j}"], it_r, Cs[j], cfg, ph=f"e{j}",
                            last_phase=(i == epc - 1))
    nc.compile()
    return nc


def _panelize_w13(w1, w3, itiles):
    """(D, I) x2 -> (itiles, 128, 2, kd, 128): one contiguous DMA per I-tile
    panel carrying both the W1 and W3 slices."""
    dd, ii = w1.shape
    p1 = w1.reshape(dd // P, P, itiles, P).transpose(2, 1, 0, 3)
    p3 = w3.reshape(dd // P, P, itiles, P).transpose(2, 1, 0, 3)
    return np.ascontiguousarray(np.stack([p1, p3], axis=2))


def prep(x, weights, indices, W1, W3, W2, Ws1, Ws3, Ws2, cfg, force_C=None):
    """Host-side dispatch: shard/gather/pad/cast/pre-tile all inputs."""
    T, D, E = cfg["T"], cfg["D"], cfg["E"]
    dv = _derived(cfg)
    epc, kd, it_r, it_s = dv["epc"], dv["kd"], dv["it_r"], dv["it_s"]
    nt, sh_tok = dv["n_tok_shards"], dv["sh_tok"]
    sh_half = cfg["sh_half"]

    xf = np.asarray(x, F32).reshape(T, D)
    wts = np.asarray(weights, F32)
    idx = np.asarray(indices).astype(np.int64)
    W1 = np.asarray(W1, F32)
    W3 = np.asarray(W3, F32)
    W2 = np.asarray(W2, F32)
    Ws1 = np.asarray(Ws1, F32)
    Ws3 = np.asarray(Ws3, F32)
    Ws2 = np.asarray(Ws2, F32)

    # Per-(token, expert) combine weight; duplicate expert ids accumulate.
    comb = np.zeros((T, E), F32)
    np.add.at(comb, (np.arange(T)[:, None], idx), wts)

    # Token dispatch (host-side all-to-all): gather token ids per expert.
    tok_lists = [np.nonzero((idx == e).any(axis=1))[0] for e in range(E)]
    counts = [len(t) for t in tok_lists]
    # Sort experts by routed-token count into the `epc` phase slots so each
    # slot's capacity is only as large as its own worst expert.
    order = np.argsort(counts)[::-1]
    eslot = order.reshape(epc, cfg["ncores"])  # eslot[j][c] = expert id
    if force_C is None:
        Cs = [int(max(NSZ, -(-max(counts[e] for e in eslot[j]) // 2) * 2))
              for j in range(epc)]
    else:
        Cs = [force_C] * epc
    for j in range(epc):
        assert Cs[j] >= max(counts[e] for e in eslot[j])

    xT16 = np.ascontiguousarray(xf.T).astype(BF16)  # (D, T)

    def _xt_layout(cols):
        # (D, n) -> (kd//XG, P, XG, n): one contiguous DMA per k-tile group.
        n = cols.shape[1]
        return np.ascontiguousarray(
            cols.reshape(kd // XG, XG, P, n).swapaxes(1, 2))

    in_maps = []
    for c in range(cfg["ncores"]):
        m = {}
        for j in range(epc):
            e = int(eslot[j][c])
            toks = tok_lists[e]
            tpad = np.zeros(Cs[j], np.int64)
            tpad[:counts[e]] = toks
            m[f"xt{j}"] = _xt_layout(xT16[:, tpad])
            m[f"w13_{j}"] = _panelize_w13(W1[e], W3[e], it_r).astype(BF16)
            m[f"w2_{j}"] = np.ascontiguousarray(
                W2[e].reshape(it_r, P, cfg["d_out"]).transpose(1, 0, 2)).astype(BF16)
            cg = np.zeros(Cs[j], F32)
            cg[:counts[e]] = comb[toks, e]
            m[f"cb{j}"] = np.ascontiguousarray(np.broadcast_to(cg, (P, Cs[j])))
        # Shared expert: 2-way inter split x (ncores/2)-way token split.
        # xts is token-halved: [g][h] = [P, XG, sh_tok/2] contiguous blocks.
        h, q = divmod(c, nt)
        xts4 = _xt_layout(xT16[:, q * sh_tok:(q + 1) * sh_tok])
        m["xts"] = np.ascontiguousarray(
            xts4.reshape(kd // XG, P, XG, 2, sh_tok // 2).transpose(0, 3, 1, 2, 4))
        m["ws13"] = _panelize_w13(Ws1[:, h * sh_half:(h + 1) * sh_half],
                                  Ws3[:, h * sh_half:(h + 1) * sh_half],
                                  it_s).astype(BF16)
        # Fast-start halves of the shared (first) phase's first two I-tile
        # panels.
        m["ws13s"] = np.ascontiguousarray(
            m["ws13"][:2].reshape(2, P, 2, 2, kd // 2, P)
            .transpose(0, 3, 1, 2, 4, 5).reshape(4, P, 2, kd // 2, P))
        m["ws2"] = np.ascontiguousarray(
            Ws2[h * sh_half:(h + 1) * sh_half].reshape(it_s, P, cfg["d_out"]).transpose(1, 0, 2)).astype(BF16)
        in_maps.append(m)

    meta = dict(tok_lists=tok_lists, counts=counts, Cs=Cs, eslot=eslot)
    return in_maps, meta


def combine(results, meta, cfg):
    """Host-side unshard: sum shared partials, scatter-add routed outputs."""
    T, D = cfg["T"], cfg["d_out"]
    dv = _derived(cfg)
    epc, nt, sh_tok = dv["epc"], dv["n_tok_shards"], dv["sh_tok"]
    kd = dv["kd"]
    out = np.zeros((T, D), F32)
    for c in range(cfg["ncores"]):
        r = results[c]
        q = c % nt
        # z: [P, kd, sh_tok] bf16 -> (D, sh_tok) with D = (kd, P); two cores
        # (inter halves) add into the same token quarter.
        z = r["z"].astype(F32).transpose(1, 0, 2).reshape(D, sh_tok)
        out[q * sh_tok:(q + 1) * sh_tok] += z.T
        for j in range(epc):
            e = int(meta["eslot"][j][c])
            yt = r[f"y{j}"].astype(F32).transpose(1, 0, 2).reshape(D, -1)
            out[meta["tok_lists"][e]] += yt.T[:meta["counts"][e]]
    return out


# Test-harness knobs (kernel() callers get no-trace defaults).
TRACE = False
TMPDIR = None
LAST_RESULT = None


def kernel(x, weights, indices, W1, W3, W2, Ws1, Ws3, Ws2):
    global LAST_RESULT
    cfg = FULL_CFG
    in_maps, meta = prep(x, weights, indices, W1, W3, W2,
                         Ws1, Ws3, Ws2, cfg)
    nc = build_program(meta["Cs"], cfg)
    res = run_bass_kernel_spmd(nc, in_maps, core_ids=list(range(cfg["ncores"])),
                               trace=TRACE, tmpdir=TMPDIR)
    LAST_RESULT = res
    out = combine(res.results, meta, cfg)
    return out.reshape(1, cfg["T"], cfg["D"]).astype(F32)
